# revision 3
# baseline (speedup 1.0000x reference)
"""Trainium2 Bass kernel for the DifferentiableQuantumCircuit problem.

Math: output = |U x / ||x|| |^2 with U = kron of 12 single-qubit U3 gates
applied twice (2 layers). Gates on different qubits commute, so the two
layers fuse into ONE kron-product unitary with per-qubit gates
G_q = U3_layer2(q) @ U3_layer1(q).

U_total = M7 (x) M5, where M7 = kron(G_0..G_6) [128x128] acts on the high
7 bits of the state index and M5 = kron(G_7..G_11) [32x32] acts on the low
5 bits.

Per-core pipeline (512 samples/core, 4 chunks of 128 samples):
  1. DMA-load chunk transposed: X[q, (b,l)] = x[b, q*32+l]  (128B bursts)
  2. stage 1: PE matmuls, stationary = X column-chunk, moving =
     [Re(M7^T) | Im(M7^T)]  -> psum[(b2,l), (re/im, p')]
     (applies M7 on the partition index AND transposes the free chunk
     (b2,l) onto partitions in one op)
  3. evacuate psum -> SBUF with 1/||x_b|| scaling fused (per-partition vec)
  4. stage 2: PE matmul pairs, stationary = S1r/S1i column-chunks,
     moving = [Re(G5^T)|Im(G5^T)] and [-Im(G5^T)|Re(G5^T)] with
     G5 = I4 (x) M5, accumulated in psum -> [p', (re/im, (b2,l'))]
  5. squares on ScalarE, re^2+im^2 add on GpSimd
  6. DMA-store probs[p', (c,b2,l')] -> out[b, i]  (128B bursts)

Norm chain: x^2 (ScalarE) -> 32-segment reduce (VectorE) -> cross-partition
sum via ones-matmul (PE) -> sqrt (ScalarE) -> reciprocal (VectorE).
"""

from contextlib import ExitStack

import numpy as np

import concourse.bass as bass
import concourse.tile as tile
from concourse import bacc, mybir
from concourse.bass_utils import run_bass_kernel_spmd

F32 = mybir.dt.float32
F32R = mybir.dt.float32r

NUM_QUBITS = 12
D = 4096
B = 4096
N_CORES = 8
B_CORE = B // N_CORES  # 512
CHUNK = 128
N_CHUNKS = B_CORE // CHUNK  # 4
NC_TILES = 32  # c-tiles per chunk (128 cols each)
GROUP = 4  # c-tiles per psum group tile (2 banks)
N_GROUPS = NC_TILES // GROUP  # 8

MM_DT = F32R  # matmul compute dtype (fp32r: fp32 bits, fast PE mode)


def _u3(theta, phi, lam):
    """Single-qubit U3 gate, complex128 [2,2] (same formula as reference)."""
    c = np.cos(theta / 2.0)
    s = np.sin(theta / 2.0)
    return np.array(
        [
            [c, -np.exp(1j * lam) * s],
            [np.exp(1j * phi) * s, np.exp(1j * (phi + lam)) * c],
        ],
        dtype=np.complex128,
    )


def _gate_consts(thetas, phis, lams):
    """Build the constant moving-operand matrices for both PE stages."""
    thetas = np.asarray(thetas, dtype=np.float64)
    phis = np.asarray(phis, dtype=np.float64)
    lams = np.asarray(lams, dtype=np.float64)
    gates = []
    for q in range(NUM_QUBITS):
        g1 = _u3(thetas[0, q], phis[0, q], lams[0, q])
        g2 = _u3(thetas[1, q], phis[1, q], lams[1, q])
        gates.append(g2 @ g1)  # layer 1 applied first, then layer 2

    m7 = gates[0]
    for q in range(1, 7):
        m7 = np.kron(m7, gates[q])  # [128,128], acts on state bits 0-6 (MSBs)
    m5 = gates[7]
    for q in range(8, 12):
        m5 = np.kron(m5, gates[q])  # [32,32], acts on state bits 7-11

    g5 = np.kron(np.eye(4), m5)  # [128,128] block-diag over (b2, l)

    mv1 = np.concatenate([m7.T.real, m7.T.imag], axis=1)  # [128,256]
    mv2a = np.concatenate([g5.T.real, g5.T.imag], axis=1)
    mv2b = np.concatenate([-g5.T.imag, g5.T.real], axis=1)
    return (
        np.ascontiguousarray(mv1, dtype=np.float32),
        np.ascontiguousarray(mv2a, dtype=np.float32),
        np.ascontiguousarray(mv2b, dtype=np.float32),
    )


def _build_nc():
    nc = bacc.Bacc(
        "TRN2", target_bir_lowering=False, debug=False, num_devices=N_CORES
    )
    x_ap = nc.dram_tensor("x", [B_CORE, D], F32R, kind="ExternalInput").ap()
    mv1_ap = nc.dram_tensor("mv1", [128, 256], F32R, kind="ExternalInput").ap()
    mv2a_ap = nc.dram_tensor("mv2a", [128, 256], F32R, kind="ExternalInput").ap()
    mv2b_ap = nc.dram_tensor("mv2b", [128, 256], F32R, kind="ExternalInput").ap()
    out_ap = nc.dram_tensor("probs", [B_CORE, D], F32, kind="ExternalOutput").ap()

    with tile.TileContext(nc) as tc, ExitStack() as ctx:
        consts = ctx.enter_context(tc.tile_pool(name="consts", bufs=1))
        mv1_t = consts.tile([128, 256], F32R, tag="mv1")
        nc.sync.dma_start(mv1_t[:], mv1_ap[:])
        mv2a_t = consts.tile([128, 256], F32R, tag="mv2a")
        nc.sync.dma_start(mv2a_t[:], mv2a_ap[:])
        mv2b_t = consts.tile([128, 256], F32R, tag="mv2b")
        nc.sync.dma_start(mv2b_t[:], mv2b_ap[:])
        ones_t = consts.tile([128, 128], F32, tag="ones")
        nc.vector.memset(ones_t[:], 1.0)

        xpool = ctx.enter_context(tc.tile_pool(name="xp", bufs=2))
        x2pool = ctx.enter_context(tc.tile_pool(name="x2p", bufs=1))
        smallp = ctx.enter_context(tc.tile_pool(name="smallp", bufs=2))
        s1pool = ctx.enter_context(tc.tile_pool(name="s1p", bufs=2))
        tpool = ctx.enter_context(tc.tile_pool(name="tp", bufs=1))
        ppool = ctx.enter_context(tc.tile_pool(name="pp", bufs=2))
        ps1 = ctx.enter_context(tc.tile_pool(name="ps1", bufs=2, space="PSUM"))
        ps2 = ctx.enter_context(tc.tile_pool(name="ps2", bufs=2, space="PSUM"))

        for k in range(N_CHUNKS):
            # ---- load chunk transposed: X[q, b*32+l] = x[k*128+b, q*32+l]
            X = xpool.tile([128, D], F32R, tag="X")
            nc.sync.dma_start(
                X[:].rearrange("q (b l) -> q b l", l=32),
                x_ap[k * CHUNK : (k + 1) * CHUNK, :].rearrange(
                    "b (q l) -> q b l", q=128
                ),
            )

            # ---- per-sample 1/||x|| vector, laid out per (b2,l) partition
            x2 = x2pool.tile([128, D], F32, tag="x2")
            nc.scalar.square(x2[:], X[:].bitcast(F32))
            sqb = smallp.tile([128, 128], F32, tag="sqb")
            nc.vector.tensor_reduce(
                sqb[:],
                x2[:].rearrange("q (b l) -> q b l", l=32),
                axis=mybir.AxisListType.X,
                op=mybir.AluOpType.add,
            )
            psv = ps1.tile([128, 128], F32, tag="g1")
            nc.tensor.matmul(
                psv[:],
                lhsT=ones_t[:],
                rhs=sqb[:].rearrange("q (c b2) -> q b2 c", b2=4),
                start=True,
                stop=True,
            )
            vecs = smallp.tile([128, 32], F32, tag="vecs")
            for b2 in range(4):
                pr = slice(b2 * 32, (b2 + 1) * 32)
                nc.scalar.sqrt(vecs[pr, :], psv[pr, b2 * 32 : (b2 + 1) * 32])
            vec = smallp.tile([128, 32], F32, tag="vec")
            nc.vector.reciprocal(vec[:], vecs[:])

            # ---- stage 1: amp1 = M7 @ x, transposing (b2,l) onto partitions
            S1r = s1pool.tile([128, D], F32R, tag="S1r")
            S1i = s1pool.tile([128, D], F32R, tag="S1i")
            for g in range(N_GROUPS):
                pg = ps1.tile([128, GROUP * 256], F32, tag="g1")
                for j in range(GROUP):
                    c = g * GROUP + j
                    nc.tensor.matmul(
                        pg[:, j * 256 : (j + 1) * 256],
                        lhsT=X[:, c * 128 : (c + 1) * 128],
                        rhs=mv1_t[:],
                        start=True,
                        stop=True,
                    )
                # evacuate with 1/||x|| scaling (scale varies per j -> use
                # a broadcast-AP tensor_tensor multiply)
                pg3 = pg[:].rearrange("p (j n) -> p j n", n=256)
                vb = (
                    vec[:, g * GROUP : (g + 1) * GROUP]
                    .unsqueeze(2)
                    .broadcast_to([128, GROUP, 128])
                )
                gcols = slice(g * GROUP * 128, (g + 1) * GROUP * 128)
                nc.vector.tensor_tensor(
                    S1r[:, gcols].rearrange("p (j n) -> p j n", n=128),
                    pg3[:, :, 0:128],
                    vb,
                    op=mybir.AluOpType.mult,
                )
                nc.vector.tensor_tensor(
                    S1i[:, gcols].rearrange("p (j n) -> p j n", n=128),
                    pg3[:, :, 128:256],
                    vb,
                    op=mybir.AluOpType.mult,
                )

            # ---- stage 2: amp2 = (I4 (x) M5) @ amp1, transposing p' onto
            # partitions; then squares
            T1 = tpool.tile([128, D], F32, tag="T1")
            T2 = tpool.tile([128, D], F32, tag="T2")
            for g in range(N_GROUPS):
                pg = ps2.tile([128, GROUP * 256], F32, tag="g2")
                for j in range(GROUP):
                    c = g * GROUP + j
                    cc = slice(c * 128, (c + 1) * 128)
                    nc.tensor.matmul(
                        pg[:, j * 256 : (j + 1) * 256],
                        lhsT=S1r[:, cc],
                        rhs=mv2a_t[:],
                        start=True,
                        stop=False,
                    )
                    nc.tensor.matmul(
                        pg[:, j * 256 : (j + 1) * 256],
                        lhsT=S1i[:, cc],
                        rhs=mv2b_t[:],
                        start=False,
                        stop=True,
                    )
                pg3 = pg[:].rearrange("p (j n) -> p j n", n=256)
                gcols = slice(g * GROUP * 128, (g + 1) * GROUP * 128)
                nc.scalar.square(
                    T1[:, gcols].rearrange("p (j n) -> p j n", n=128),
                    pg3[:, :, 0:128],
                )
                nc.scalar.square(
                    T2[:, gcols].rearrange("p (j n) -> p j n", n=128),
                    pg3[:, :, 128:256],
                )

            # ---- probs = re^2 + im^2 ; store transposed back to [b, i]
            P = ppool.tile([128, D], F32, tag="P")
            nc.gpsimd.tensor_tensor(
                P[:], T1[:], T2[:], op=mybir.AluOpType.add
            )
            nc.sync.dma_start(
                out_ap[k * CHUNK : (k + 1) * CHUNK, :].rearrange(
                    "(c b2) (p l) -> p c b2 l", b2=4, l=32
                ),
                P[:].rearrange("p (c b2 l) -> p c b2 l", b2=4, l=32),
            )

    nc.compile()
    return nc


_NC_CACHE = {}


def _get_nc():
    if "nc" not in _NC_CACHE:
        _NC_CACHE["nc"] = _build_nc()
    return _NC_CACHE["nc"]


def kernel(inputs, thetas, phis, lams, _trace=False, _trace_kwargs=None):
    inputs = np.ascontiguousarray(np.asarray(inputs), dtype=np.float32)
    mv1, mv2a, mv2b = _gate_consts(thetas, phis, lams)

    nc = _get_nc()
    in_maps = [
        {
            "x": inputs[k * B_CORE : (k + 1) * B_CORE],
            "mv1": mv1,
            "mv2a": mv2a,
            "mv2b": mv2b,
        }
        for k in range(N_CORES)
    ]
    res = run_bass_kernel_spmd(
        nc, in_maps, list(range(N_CORES)), trace=_trace, **(_trace_kwargs or {})
    )
    out = np.concatenate([res.results[k]["probs"] for k in range(N_CORES)], axis=0)
    if _trace:
        kernel.last_result = res
    return out
